# revision 21
# baseline (speedup 1.0000x reference)
"""Causal self-attention (B=2, T=4096, C=768, NH=12) on 8 trn2 cores.

Sharding: data-parallel over batch (2) x tensor-parallel over heads (12 -> 3
heads/core).  Core c handles batch c//4, heads 3*(c%4) .. 3*(c%4)+2.  Each
core computes qkv for its heads, causal attention, and its partial of the
output projection; a 4-core bf16 ReduceScatter per 256-row chunk reduces the
partials, each core keeps 64 rows per chunk, and the host reassembles.

v2 design (ACT-paced):
 - x arrives pre-transposed/bf16 from the host (xT) - no PE transposes, no
   fp32 x loads, no DVE transpose copies.
 - QK for heads 0/1 run as one row-tiled concurrent pair (K=64 tiles at
   array rows 0-63 / 64-127); head 2 pairs (j, j+1) the same way via
   duplicated qT3/kT3 partition halves.
 - exp: 2 ACT calls per kv tile (heads01 [128,1024] + head2 [128,512], both
   PSUM-sourced); ACT is the pacing engine (~1.57us per kv tile).
 - PV per head is one M=65 [v|ones] matmul (denominator rides the 65th
   column); each head's accumulator owns a full PSUM bank (one accumulation
   group per bank - hardware start=True zeroes whole-bank has_written).
 - softmax reciprocal via DVE reciprocal_approx_fast - no Ln, so the ACT
   exp table set is loaded exactly once (baseline paid 49 table loads =
   63us of ACT).
 - PSUM banks: scores 4 (sc01 bufs=1 + sc2 bufs=2) + PV 3 + scratch 1 = 8.
 - 16 ReduceScatter chunks to shrink the collective tail.
"""

import sys

if "/opt/trn_rl_repo" not in sys.path:
    sys.path.insert(0, "/opt/trn_rl_repo")

import numpy as np

B, T, C = 2, 4096, 768
NH, HD = 12, 64
N_CORES = 8
HPC = 3  # heads per core
TB = 512  # q block size
KT = 128  # kv tile size
NQB = T // TB  # 8 q blocks
NTT = T // KT  # 32 kv tiles
NCH = C // 128  # 6 contraction chunks
NCHUNK = 8  # reduce-scatter chunks (one per q block)
CH_ROWS = T // NCHUNK  # 512
SH_ROWS = CH_ROWS // 4  # 128 rows per core per chunk
# RS chunk layout (in_row0, n_rows): q-block rows for qb1..7, qb0 (processed
# last) split small to shrink the tail.  cc_out offset = in_row0 // 4.
RS_LAYOUT = [(qb * TB, TB) for qb in range(1, NQB)] + \
            [(0, 256), (256, 128), (384, 128)]
VW = HPC * 65 + 1  # 196: per-j vsb slot [v0|1|v1|1|v2|1|pad]
SCALE = float(HD) ** -0.5

_CACHE = {}


def _build():
    if "nc" in _CACHE:
        return _CACHE["nc"]

    from concourse import bacc, tile, mybir

    dt = mybir.dt
    ActFn = mybir.ActivationFunctionType
    Alu = mybir.AluOpType

    nc = bacc.Bacc("TRN2", target_bir_lowering=False, debug=False,
                   num_devices=N_CORES)

    xt_in = nc.dram_tensor("xt", [NQB, NCH, 128, TB], dt.bfloat16,
                           kind="ExternalInput")
    wqk_in = nc.dram_tensor("wqk", [C, HPC * 128], dt.bfloat16, kind="ExternalInput")
    wv_in = nc.dram_tensor("wv", [C, VW], dt.bfloat16, kind="ExternalInput")
    wp_in = nc.dram_tensor("wp", [64, HPC * C], dt.bfloat16, kind="ExternalInput")
    bqk_in = nc.dram_tensor("bqk", [128, HPC], dt.float32, kind="ExternalInput")
    bv_in = nc.dram_tensor("bv", [1, VW], dt.bfloat16, kind="ExternalInput")
    tmask_in = nc.dram_tensor("tmask", [128, 128], dt.bfloat16, kind="ExternalInput")
    out_ext = nc.dram_tensor("out", [NCHUNK * SH_ROWS, C], dt.bfloat16,
                             kind="ExternalOutput")

    groups = [[0, 1, 2, 3], [4, 5, 6, 7]]

    with tile.TileContext(nc) as tc:
        with (
            tc.tile_pool(name="persist", bufs=1) as pp,
            tc.tile_pool(name="dram", bufs=1, space="DRAM") as dp,
            tc.tile_pool(name="ptp", bufs=4) as ptp,
            tc.tile_pool(name="stp", bufs=2) as stp,
            tc.tile_pool(name="ytp", bufs=2) as ytp,
            tc.tile_pool(name="dnp", bufs=2) as dnp,
            tc.tile_pool(name="bcp", bufs=2) as bcp,
            tc.tile_pool(name="ynp", bufs=2) as ynp,
            tc.tile_pool(name="osp", bufs=3) as osp,
            tc.tile_pool(name="psS", bufs=1, space="PSUM") as psS,
            tc.tile_pool(name="psY", bufs=3, space="PSUM") as psY,
            tc.tile_pool(name="psB", bufs=1, space="PSUM") as psB,
        ):
            # ---- persistent SBUF ----
            xT = pp.tile([128, NCH, T], dt.bfloat16, tag="xT")
            qT2 = pp.tile([128, T], dt.bfloat16, tag="qT2")  # h0 | h1
            kT2 = pp.tile([128, T], dt.bfloat16, tag="kT2")
            qT3 = pp.tile([128, T], dt.bfloat16, tag="qT3")  # h2 dup halves
            kT3 = pp.tile([128, T], dt.bfloat16, tag="kT3")
            vsb = pp.tile([128, NTT, VW], dt.bfloat16, tag="vsb")
            wqk = pp.tile([128, NCH, HPC * 128], dt.bfloat16, tag="wqk")
            wv = pp.tile([128, NCH, VW], dt.bfloat16, tag="wv")
            wp = pp.tile([64, HPC * C], dt.bfloat16, tag="wp")
            bqk = pp.tile([128, HPC], dt.float32, tag="bqk")
            bv = pp.tile([1, VW], dt.bfloat16, tag="bv")
            ones = pp.tile([1, 128], dt.bfloat16, tag="ones")
            tmask = pp.tile([128, 128], dt.bfloat16, tag="tmask")

            cc_in = dp.tile([T, C], dt.bfloat16, tag="cc_in")
            cc_out = dp.tile([NCHUNK * SH_ROWS, C], dt.bfloat16, tag="cc_out")

            # ---- constants / weights ----
            nc.sync.dma_start(tmask[:], tmask_in.ap()[:])
            nc.sync.dma_start(bqk[:], bqk_in.ap()[:])
            nc.sync.dma_start(bv[:], bv_in.ap()[:])
            nc.vector.memset(ones[:], 1.0)
            for ci in range(NCH):
                nc.sync.dma_start(wqk[:, ci, :],
                                  wqk_in.ap()[ci * 128:(ci + 1) * 128, :])
                nc.sync.dma_start(wv[:, ci, :],
                                  wv_in.ap()[ci * 128:(ci + 1) * 128, :])
            nc.sync.dma_start(wp[:], wp_in.ap()[:])
            # x loads, block-major so early blocks land first
            for tb in range(NQB):
                for ci in range(NCH):
                    nc.sync.dma_start(xT[:, ci, tb * TB:(tb + 1) * TB],
                                      xt_in.ap()[tb, ci])

            # ---- scratch psum rotation (phase-0 may use score banks) ----
            def scratch_ring():
                slots = [(psB, "b", [128, 512]),
                         (psS, "sc2", [128, 512]),
                         (psS, "sc2", [128, 512]),
                         (psS, "sc01", [128, 1024])]
                i = 0
                while True:
                    pool, tag, shp = slots[i % len(slots)]
                    i += 1
                    yield pool, tag, shp

            _ring = scratch_ring()

            def scratch_phase0():
                pool, tag, shp = next(_ring)
                return pool.tile(shp, dt.float32, tag=tag, name="scr")

            def scratch_attn():
                return psB.tile([128, 512], dt.float32, tag="b", name="scr")

            # ---- qkv-gen thunks ----
            def thunk_D(hh, tb, scratch):
                def run():
                    tsl = slice(tb * TB, (tb + 1) * TB)
                    pq = scratch()
                    for ci in range(NCH):
                        nc.tensor.matmul(
                            pq[:, 0:TB],
                            wqk[:, ci, hh * 128:(hh + 1) * 128],
                            xT[:, ci, tsl],
                            start=(ci == 0), stop=(ci == NCH - 1))
                    st = stp.tile([128, TB], dt.bfloat16, tag="qkst")
                    nc.vector.tensor_scalar_add(st[:], pq[:, 0:TB],
                                                bqk[:, hh:hh + 1])
                    if hh < 2:
                        nc.gpsimd.dma_start(qT2[hh * 64:(hh + 1) * 64, tsl],
                                            st[0:64, :])
                        nc.gpsimd.dma_start(kT2[hh * 64:(hh + 1) * 64, tsl],
                                            st[64:128, :])
                    else:
                        nc.gpsimd.dma_start(qT3[0:64, tsl], st[0:64, :])
                        nc.gpsimd.dma_start(qT3[64:128, tsl], st[0:64, :])
                        nc.gpsimd.dma_start(kT3[0:64, tsl], st[64:128, :])
                        nc.gpsimd.dma_start(kT3[64:128, tsl], st[64:128, :])
                return run

            def thunk_C(j, scratch):
                def run():
                    pv = scratch()
                    for ci in range(NCH):
                        nc.tensor.matmul(
                            pv[:, 0:VW], xT[:, ci, j * 128:(j + 1) * 128],
                            wv[:, ci, :], start=(ci == 0), stop=False)
                    # ones columns ride the bias row (bv[h*65+64] = 1)
                    nc.tensor.matmul(pv[:, 0:VW], ones[:], bv[:],
                                     start=False, stop=True)
                    nc.vector.tensor_copy(vsb[:, j, :], pv[:, 0:VW])
                return run

            # ---- fill lists ----
            thunk_fill = []   # deadline: tb thunks before attention(tb)
            soft_fill = []    # proj units, no hard deadline
            state = {"thunk_drained": 0}

            def drain_thunk(k):
                for _ in range(min(k, len(thunk_fill))):
                    thunk_fill.pop(0)()
                    state["thunk_drained"] += 1

            def drain_soft(k):
                for _ in range(min(k, len(soft_fill))):
                    soft_fill.pop(0)()

            # ---- normalize + projection ----
            def norm(qb, yT, dnc):
                # 1/d on DVE (exact iterative divide, ~3.3us per call but on
                # the non-critical engine; approx_fast computes garbage on HW
                # and Ln/Exp would thrash ACT table sets).  Reads the SBUF
                # den copy so the psum banks are already free.
                dnr = dnp.tile([65, HPC, TB], dt.float32, tag="dnr")
                for h in range(HPC):
                    nc.vector.reciprocal(dnr[64:65, h, :],
                                         dnc[64:65, h, :])
                dnb = dnp.tile([65, HPC, TB], dt.bfloat16, tag="dnb")
                nc.vector.tensor_copy(dnb[64:65, :, :], dnr[64:65, :, :])
                sbb = bcp.tile([64, HPC, TB], dt.bfloat16, tag="sbb")
                for h in range(HPC):
                    nc.gpsimd.dma_start(
                        sbb[:, h, :],
                        dnb[64:65, h, :].unsqueeze(1).broadcast_to([1, 64, TB]))
                yN = ynp.tile([64, HPC, TB], dt.bfloat16, tag="yN")
                nc.vector.tensor_tensor(yN[:], yT[:], sbb[:], op=Alu.mult)
                return yN

            yns = [None] * NQB
            # RS chunk table: (qb, qs_that_completes_it, in_row0, n_rows).
            # qb0 runs last, so its chunks are small to shrink the tail.
            RS_CHUNKS = [(qb, 3, qb * TB, TB) for qb in range(1, NQB)]
            RS_CHUNKS += [(0, 1, 0, 256), (0, 2, 256, 128), (0, 3, 384, 128)]

            def proj_unit(qb, qs, scratch=None):
                def run():
                    yN = yns[qb]
                    csl = slice(qs * 128, (qs + 1) * 128)
                    osb = osp.tile([128, C], dt.bfloat16, tag="osb")
                    for half in range(2):
                        if scratch is None:
                            pb = psB.tile([128, 512], dt.float32, tag="b",
                                          name="pb")
                        else:
                            pb = scratch()
                        for h in range(HPC):
                            nc.tensor.matmul(
                                pb[:, 0:384], yN[0:64, h, csl],
                                wp[:, h * C + half * 384: h * C + half * 384 + 384],
                                start=(h == 0), stop=(h == HPC - 1))
                        nc.vector.tensor_copy(
                            osb[:, half * 384:half * 384 + 384], pb[:, 0:384])
                    r0 = qb * TB + qs * 128
                    nc.gpsimd.dma_start(cc_in[r0:r0 + 128, :], osb[:])
                    for cqb, cqs, cr0, crn in RS_CHUNKS:
                        if cqb == qb and cqs == qs:
                            nc.gpsimd.collective_compute(
                                "ReduceScatter", Alu.add,
                                replica_groups=groups,
                                ins=[cc_in[cr0:cr0 + crn, :]],
                                outs=[cc_out[cr0 // 4:(cr0 + crn) // 4, :]])
                return run

            # ---- attention ----
            def attention(qb, thunk_target):
                n_kv = 4 * (qb + 1)
                diag0 = 4 * qb
                qsl = slice(qb * TB, (qb + 1) * TB)
                pys = [psY.tile([65, TB], dt.float32, tag="py",
                                name=f"py{h}") for h in range(HPC)]
                pts = [None] * n_kv
                s2s = [None] * n_kv

                def emit_masks(pt, hh, k0):
                    if k0 > 0:
                        nc.vector.memset(pt[:, hh, 0:k0], 0.0)
                    nc.vector.tensor_mul(pt[:, hh, k0:k0 + KT],
                                         pt[:, hh, k0:k0 + KT], tmask[:])

                def emit_pv(j, first, last):
                    pt = pts[j]
                    for h in range(HPC):
                        nc.tensor.matmul(
                            pys[h][:], vsb[:, j, h * 65:(h + 1) * 65],
                            pt[:, h, :], start=first, stop=last)

                for j in range(n_kv):
                    jsl = slice(j * KT, (j + 1) * KT)
                    k0 = (j - diag0) * KT if j >= diag0 else -1

                    # QK heads 0/1: row-tiled concurrent pair
                    s01 = psS.tile([128, 2, TB], dt.float32, tag="sc01",
                                   name="s01")
                    nc.tensor.matmul(s01[:, 0, :], kT2[0:64, jsl],
                                     qT2[0:64, qsl], start=True, stop=True)
                    nc.tensor.matmul(s01[:, 1, :], kT2[64:128, jsl],
                                     qT2[64:128, qsl], start=True, stop=True)
                    # QK head 2: (j, j+1) row-tiled pair
                    if j % 2 == 0:
                        j1sl = slice((j + 1) * KT, (j + 2) * KT)
                        s2a = psS.tile([128, TB], dt.float32, tag="sc2",
                                       name="s2a")
                        s2b = psS.tile([128, TB], dt.float32, tag="sc2",
                                       name="s2b")
                        nc.tensor.matmul(s2a[:], kT3[0:64, jsl],
                                         qT3[0:64, qsl], start=True, stop=True)
                        nc.tensor.matmul(s2b[:], kT3[64:128, j1sl],
                                         qT3[64:128, qsl], start=True,
                                         stop=True)
                        s2s[j] = s2a
                        s2s[j + 1] = s2b

                    pt = ptp.tile([128, HPC, TB], dt.bfloat16, tag="pt")
                    pts[j] = pt
                    # exp: heads01 then head2
                    if k0 <= 0:
                        nc.scalar.activation(pt[:, 0:2, :], s01[:],
                                             ActFn.Exp, scale=SCALE)
                        nc.scalar.activation(pt[:, 2, :], s2s[j][:],
                                             ActFn.Exp, scale=SCALE)
                        if k0 == 0:
                            for hh in range(HPC):
                                emit_masks(pt, hh, 0)
                    else:
                        nc.scalar.activation(
                            pt[:, 0:2, k0:TB], s01[:, :, k0:TB],
                            ActFn.Exp, scale=SCALE)
                        nc.scalar.activation(pt[:, 2, k0:TB],
                                             s2s[j][:, k0:TB],
                                             ActFn.Exp, scale=SCALE)
                        for hh in range(HPC):
                            emit_masks(pt, hh, k0)

                    if j >= 1:
                        emit_pv(j - 1, j - 1 == 0, False)

                    # paced filler drain: thunk units have per-block
                    # deadlines; proj units drain in the late half of the
                    # block so they don't head-block the PE queue behind the
                    # previous block's reciprocal chain
                    need = thunk_target - state["thunk_drained"]
                    left = n_kv - j
                    if need >= left and need > 0:
                        drain_thunk((need + left - 1) // left)
                    elif need > 0 and j % 2 == 0:
                        drain_thunk(1)
                    elif j % 2 == 1 and j >= max(3, n_kv // 2) and soft_fill:
                        drain_soft(1)

                emit_pv(n_kv - 1, n_kv - 1 == 0, True)

                # evacuate psum on ACT (Identity shares the exp table set and
                # ACT idles at block boundaries anyway); dens first so the
                # slow DVE reciprocals run off SBUF with the py banks free
                dnc = dnp.tile([65, HPC, TB], dt.float32, tag="dnc")
                for h in range(HPC):
                    nc.scalar.activation(dnc[64:65, h, :], pys[h][64:65, :],
                                         ActFn.Identity)
                yT = ytp.tile([64, HPC, TB], dt.bfloat16, tag="yT")
                for h in range(HPC):
                    nc.scalar.activation(yT[0:64, h, :], pys[h][0:64, :],
                                         ActFn.Identity)
                yns[qb] = norm(qb, yT, dnc)

            # ---- schedule ----
            # q-blocks processed [1..7, 0]: the last block's tail chain
            # (attention -> norm -> proj -> final RS) is then the shortest.
            order = [1, 2, 3, 4, 5, 6, 7, 0]
            for tb in (0, 1):
                for hh in range(HPC):
                    thunk_D(hh, tb, scratch_phase0)()
            for j in range(8):
                thunk_C(j, scratch_phase0)()
            for tb in range(2, NQB):
                for hh in range(HPC):
                    thunk_fill.append(thunk_D(hh, tb, scratch_attn))
                for j in range(4 * tb, 4 * tb + 4):
                    thunk_fill.append(thunk_C(j, scratch_attn))

            def units_needed(qb):
                # fill units (tb>=2) that must be drained before attention(qb)
                return 7 * max(0, qb - 1)

            for idx, qb in enumerate(order):
                drain_thunk(units_needed(qb) - state["thunk_drained"])
                nxt = order[idx + 1] if idx + 1 < len(order) else None
                tgt = units_needed(nxt) if nxt is not None else len(thunk_fill)
                attention(qb, tgt)
                if qb != order[-1]:
                    for qs in range(4):
                        soft_fill.append(proj_unit(qb, qs))
            drain_thunk(len(thunk_fill))
            drain_soft(len(soft_fill))
            # tail proj units rotate through the now-free score banks
            for qs in range(4):
                proj_unit(order[-1], qs, scratch_phase0)()
            # final copy at the end: its RS-completion waits block nothing
            nc.sync.dma_start(out_ext.ap()[:], cc_out[:])

    nc.compile()
    _CACHE["nc"] = nc
    return nc


def _prep_core_inputs(x, w_attn, b_attn, w_proj, b_proj):
    """Host-side sharding: returns list of 8 input dicts."""
    import ml_dtypes

    bf16 = ml_dtypes.bfloat16
    tmask = np.triu(np.ones((128, 128), np.float32)).astype(bf16)
    in_maps = []
    for core in range(N_CORES):
        b = core // 4
        h0 = HPC * (core % 4)
        wqk = np.empty((C, HPC * 128), np.float32)
        bqk = np.empty((128, HPC), np.float32)
        wv = np.zeros((C, VW), np.float32)
        bv = np.zeros((1, VW), np.float32)
        wp = np.empty((64, HPC * C), np.float32)
        for hh in range(HPC):
            h = h0 + hh
            wqk[:, hh * 128: hh * 128 + 64] = w_attn[:, h * HD:(h + 1) * HD]
            wqk[:, hh * 128 + 64: hh * 128 + 128] = w_attn[:, C + h * HD: C + (h + 1) * HD]
            bqk[0:64, hh] = b_attn[h * HD:(h + 1) * HD]
            bqk[64:128, hh] = b_attn[C + h * HD: C + (h + 1) * HD]
            wv[:, hh * 65:(hh + 1) * 65 - 1] = w_attn[:, 2 * C + h * HD: 2 * C + (h + 1) * HD]
            bv[0, hh * 65:(hh + 1) * 65 - 1] = b_attn[2 * C + h * HD: 2 * C + (h + 1) * HD]
            bv[0, (hh + 1) * 65 - 1] = 1.0  # ones column for the denominator
            wp[:, hh * C:(hh + 1) * C] = w_proj[h * HD:(h + 1) * HD, :]
        # xt: [NQB, NCH, 128, TB]; xt[tb,ci,p,q] = x[b][tb*TB+q, ci*128+p]
        xb = x[b].astype(bf16)  # [T, C]
        xt = np.ascontiguousarray(
            xb.reshape(NQB, TB, NCH, 128).transpose(0, 2, 3, 1))
        in_maps.append({
            "xt": xt,
            "wqk": wqk.astype(bf16),
            "wv": wv.astype(bf16),
            "wp": wp.astype(bf16),
            "bqk": bqk,
            "bv": bv.astype(bf16),
            "tmask": tmask,
        })
    return in_maps


def kernel(x, w_attn, b_attn, w_proj, b_proj, _trace=False, _trace_kwargs=None):
    x = np.asarray(x, np.float32)
    w_attn = np.asarray(w_attn, np.float32)
    b_attn = np.asarray(b_attn, np.float32)
    w_proj = np.asarray(w_proj, np.float32)
    b_proj = np.asarray(b_proj, np.float32)

    nc = _build()
    from concourse.bass_utils import run_bass_kernel_spmd

    in_maps = _prep_core_inputs(x, w_attn, b_attn, w_proj, b_proj)
    kw = dict(_trace_kwargs or {})
    res = run_bass_kernel_spmd(nc, in_maps, core_ids=list(range(N_CORES)),
                               trace=_trace, **kw)
    # reassemble: for RS chunk (r0, n), core 4*b + r holds global rows
    # r0 + r*(n/4) .. +n/4 in its out[r0/4 : r0/4 + n/4]
    out = np.empty((B, T, C), np.float32)
    bp = b_proj[None, :]
    for b in range(B):
        for r in range(4):
            o = np.asarray(res.results[4 * b + r]["out"], np.float32)
            for r0, n in RS_LAYOUT:
                sh = n // 4
                out[b, r0 + r * sh: r0 + (r + 1) * sh] = \
                    o[r0 // 4: r0 // 4 + sh] + bp
    if _trace:
        kernel.last_results = res
    return out
